# revision 64
# baseline (speedup 1.0000x reference)
"""v4: ACT-saturated schedule around the exp stream.
174396 ns (v3 baseline: 275093 ns), rel err 4.1e-03.

The softmax exp is the hard floor: B*H*S^2/8 = 16.8M elements through the
one ACT engine = 133us busy; PE matmul work lands at ~140us after the ctx
swap, so both engines are near-saturated and the schedule's job is to keep
the exp stream dense while weaving every other op into its shadow.

Cost-model-driven redesign vs v3 (see kernel_v3_baseline.py):
 - matmul cost = N(out free) x cycles_per_row(moving dtype); bf16 moving is
   1 cyc/row at any N (f32r needs N>=256). All HBM-sourced operands are
   pre-converted to bf16 on the host (halves load DMA too).
 - ctx matmul swapped: stationary = exp tile [keys,128q] (full 128x128),
   moving = V [keys, 64+ones] -> ctx cost halves; softmax denominator rides
   along as a ones column; normalization becomes a per-partition
   tensor_scalar at evac time.
 - V is projected directly transposed (stationary = qt tile, moving = wv):
   no PE transposes anywhere.
 - ctx^T for the out-projection via DMA-transpose (16x128 XBAR tiles).
 - k-bias dropped (exactly cancels in softmax), v-bias and out-bias folded
   on the host (attention rows sum to 1), q-bias folded into the QT evac.
 - ACT engine does nothing but the 128 exps (the roofline: ~133us); PE work
   of adjacent phases (proj, u1-ctx pass, outproj) is woven between score
   matmuls as cost-bounded inserts so the exp stream never starves. PE
   warmup matmuls defeat the p-state ramp.
 - PSUM (8 banks exactly): sA,sB [128,1024] (2+2), cA,cB [128,260] (1+1,
   u0 ctx: four 65-col qt groups each), w1,w2 [128,512] (1+1, rotating:
   warmup, k/q-proj chunks, v-proj tiles, u1-ctx qt groups, outproj halves).
"""

import functools
from collections import deque
from contextlib import ExitStack

import numpy as np
import ml_dtypes

import concourse.bass as bass
import concourse.tile as tile
from concourse import mybir
from concourse.bass_utils import run_bass_kernel_spmd

B, S, D, H, DH = 2, 2048, 1024, 16, 64
N_CORES = 8
DPC = D // N_CORES          # 128 channels/core = 2 heads
BS = B * S
NST = 16                    # key tiles of 128
NKT = 8                     # contraction tiles of 128

F32 = mybir.dt.float32
F32R = mybir.dt.float32r
BF16 = mybir.dt.bfloat16
Act = mybir.ActivationFunctionType
Alu = mybir.AluOpType
BF = ml_dtypes.bfloat16


def _split_sync_commands(nc, max_waits=1, max_updates=8):
    for fn in nc.m.functions:
        for bb in fn.blocks:
            new_insts = []
            changed = False
            for inst in bb.instructions:
                si = getattr(inst, "sync_info", None)
                if si is not None:
                    waits = list(si.on_wait or [])
                    if len(waits) > max_waits:
                        for w in waits[:-max_waits]:
                            new_insts.append(mybir.InstNoOp(
                                name=nc.get_next_instruction_name(),
                                ins=[], outs=[], engine=inst.engine,
                                sync_info=mybir.SyncInfo(on_wait=[w], on_update=[]),
                            ))
                        si.on_wait = waits[-max_waits:]
                        changed = True
                    updates = list(si.on_update or [])
                    if len(updates) > max_updates:
                        si.on_update = updates[:max_updates]
                        new_insts.append(inst)
                        new_insts.append(mybir.InstNoOp(
                            name=nc.get_next_instruction_name(),
                            ins=[], outs=[], engine=inst.engine,
                            sync_info=mybir.SyncInfo(
                                on_wait=[], on_update=updates[max_updates:]),
                        ))
                        changed = True
                        continue
                new_insts.append(inst)
            if changed:
                bb.instructions = new_insts


@functools.lru_cache(maxsize=1)
def _build():
    nc = bass.Bass()
    qt_d = nc.dram_tensor("qt", [D, BS], BF16, kind="ExternalInput")
    wq_d = nc.dram_tensor("wq", [128, NKT * DPC], BF16, kind="ExternalInput")
    wk_d = nc.dram_tensor("wk", [128, NKT * DPC], BF16, kind="ExternalInput")
    wv_d = nc.dram_tensor("wv", [128, NKT * DPC], BF16, kind="ExternalInput")
    bq_d = nc.dram_tensor("bq", [DPC, 1], F32, kind="ExternalInput")
    wo_d = nc.dram_tensor("wo", [DPC, D], BF16, kind="ExternalInput")
    out_d = nc.dram_tensor("out_part", [BS, D], BF16, kind="ExternalOutput")

    with tile.TileContext(nc) as tc, ExitStack() as ctx:
        consts = ctx.enter_context(tc.tile_pool(name="consts", bufs=1))
        qtp = ctx.enter_context(tc.tile_pool(name="qtp", bufs=1))
        proj = ctx.enter_context(tc.tile_pool(name="proj", bufs=2))
        vp = ctx.enter_context(tc.tile_pool(name="vp", bufs=2))
        expp = ctx.enter_context(tc.tile_pool(name="expp", bufs=1))
        csbp = ctx.enter_context(tc.tile_pool(name="csbp", bufs=8))
        ctp = ctx.enter_context(tc.tile_pool(name="ctp", bufs=2))
        rcpp = ctx.enter_context(tc.tile_pool(name="rcpp", bufs=4))
        outp = ctx.enter_context(tc.tile_pool(name="outp", bufs=3))
        psp = ctx.enter_context(tc.tile_pool(name="psp", bufs=1, space="PSUM"))

        def ps_tile(shape, tag):
            return psp.tile(shape, F32, tag=tag, name="ps_" + tag)

        _wrot = [0]

        def next_w():
            _wrot[0] ^= 1
            return "w1" if _wrot[0] else "w2"

        # ---------------- constants / warmup ----------------
        wconst = consts.tile([128, 512], BF16, tag="wconst")
        nc.vector.memset(wconst, 0.0)
        zero_sb = consts.tile([128, 1], F32, tag="zero")
        nc.vector.memset(zero_sb, 0.0)
        eighth_sb = consts.tile([128, 1], F32, tag="eighth")
        nc.vector.memset(eighth_sb, 0.125)

        for _ in range(11):
            ps = ps_tile([128, 512], next_w())
            nc.tensor.matmul(ps[:, 0:384], wconst[:, 0:128], wconst[:, 128:512],
                             start=True, stop=True)

        # ---------------- weight / input loads ----------------
        # SP queue / DMA-device order = priority order: wk, qt-b0-c0, wq,
        # qt-c1, wv, qt-c2/3, qt-b1. bq/wo ride the ACT queue.
        wk_sb = consts.tile([128, NKT, DPC], BF16, tag="wk")
        wq_sb = consts.tile([128, NKT, DPC], BF16, tag="wq")
        wv_sb = consts.tile([128, NKT, DPC], BF16, tag="wv")
        bq_sb = consts.tile([128, 1], F32, tag="bq")
        wo_sb = consts.tile([128, D], BF16, tag="wo")
        ident_d = nc.inline_tensor(
            np.eye(128, dtype=np.float32).astype(ml_dtypes.bfloat16), "identb")
        ident_sb = consts.tile([128, 128], BF16, tag="ident")

        state = {}

        def qt_chunk(b, c0, ncol=512):
            qt_sb = state[b, "qt"]
            qa = qt_d[:, :]
            nc.sync.dma_start(
                out=qt_sb[:, :, c0:c0 + ncol],
                in_=bass.AP(tensor=qa.tensor,
                            offset=qa.offset + b * S + c0,
                            ap=[[BS, 128], [128 * BS, NKT], [1, ncol]]))

        def alloc_qt(b):
            state[b, "qt"] = qtp.tile([128, NKT, S], BF16, tag=f"qt{b}",
                                      name=f"qt{b}")

        # V layout: [keys, st, 2*65]; cols u*65..u*65+63 = V_u, col u*65+64 = 1
        def alloc_v(b):
            V = vp.tile([128, NST, 130], BF16, tag="V", name="V")
            ones_ap = bass.AP(tensor=V.tensor, offset=V.offset + 64,
                              ap=[list(V.ap[0]), [130, NST], [65, 2], [1, 1]])
            nc.gpsimd.memset(ones_ap, 1.0)
            state[b, "V"] = V

        def alloc_proj(b):
            state[b, "QT"] = proj.tile([128, S], F32R, tag="QT", name="QT")
            state[b, "KT"] = proj.tile([128, S], F32R, tag="KT", name="KT")

        def kq_chunk(b, which, c, wtag, klo=0, khi=NKT, c0=None, ncol=512):
            """proj chunk (k-range part); evac on DVE at khi==NKT."""
            qt_sb = state[b, "qt"]
            w_sb = wk_sb if which == "k" else wq_sb
            dst = state[b, "KT" if which == "k" else "QT"]
            if c0 is None:
                c0 = c * 512
            sl = slice(c0, c0 + ncol)
            if klo == 0:
                state[b, "kqps", which] = ps_tile([128, 512], wtag)
            ps = state[b, "kqps", which]
            for k in range(klo, khi):
                nc.tensor.matmul(ps[:, 0:ncol], w_sb[:, k, :], qt_sb[:, k, sl],
                                 start=(k == 0), stop=(k == NKT - 1))
            if khi == NKT:
                if which == "q":
                    nc.vector.tensor_scalar(
                        out=dst[:, sl], in0=ps[:, 0:ncol], scalar1=bq_sb,
                        scalar2=eighth_sb, op0=Alu.add, op1=Alu.mult)
                else:
                    nc.vector.tensor_copy(dst[:, sl], ps[:, 0:ncol])

        def v_st(b, st, wtag):
            """v-proj directly transposed: out [bs128, dpc128]."""
            qt_sb = state[b, "qt"]
            V = state[b, "V"]
            ps = ps_tile([128, 512], wtag)
            sl = slice(st * 128, (st + 1) * 128)
            for k in range(NKT):
                nc.tensor.matmul(ps[:, 0:128], qt_sb[:, k, sl], wv_sb[:, k, :],
                                 start=(k == 0), stop=(k == NKT - 1))
            for u in range(2):
                nc.vector.tensor_copy(V[:, st, u * 65:u * 65 + 64],
                                      ps[:, u * 64:(u + 1) * 64])

        def alloc_attn(b):
            state[b, "ctxT"] = ctp.tile([128, S], BF16, tag="ctxT", name="ctxT")

        def outproj_st(b, st, wtagA, wtagB, split_evac=False):
            # adjacent st pairs share one [128, 2, 1024] tile and one store
            # DMA over 256 contiguous DRAM rows (halves Pool SWDGE issue cost)
            ctxT = state[b, "ctxT"]
            g = st % 2
            if g == 0:
                state[b, "opair"] = outp.tile([128, 2, D], BF16, tag="o",
                                              name="o_sb")
            o_sb = state[b, "opair"]
            for oc, wtag in ((0, wtagA), (1, wtagB)):
                ps = ps_tile([128, 512], wtag)
                nc.tensor.matmul(ps, ctxT[:, st * 128:(st + 1) * 128],
                                 wo_sb[:, oc * 512:(oc + 1) * 512],
                                 start=True, stop=True)
                if split_evac and oc == 1:
                    # ACT is idle after the last exp; GPSIMD can't read PSUM
                    nc.scalar.activation(o_sb[:, g, oc * 512:(oc + 1) * 512],
                                         ps, Act.Copy, bias=0.0, scale=1.0)
                else:
                    nc.vector.tensor_copy(o_sb[:, g, oc * 512:(oc + 1) * 512],
                                          ps)
            if g == 1:
                r0 = b * S + (st - 1) * 128
                oa = out_d[r0:r0 + 256, :]
                if split_evac and st == NST - 1:
                    # last pair: two singles on separate queues to shorten
                    # the final DMA drain
                    nc.gpsimd.dma_start(out=out_d[r0:r0 + 128, :],
                                        in_=o_sb[:, 0, :])
                    nc.sync.dma_start(out=out_d[r0 + 128:r0 + 256, :],
                                      in_=o_sb[:, 1, :])
                else:
                    nc.gpsimd.dma_start(
                        out=bass.AP(tensor=oa.tensor, offset=oa.offset,
                                    ap=[[D, 128], [128 * D, 2], [1, D]]),
                        in_=o_sb)

        def attention_qc(b, qc, inserts, final=False, warm_cb=None):
            QT, KT, V = state[b, "QT"], state[b, "KT"], state[b, "V"]
            ctxT = state[b, "ctxT"]
            inserts = deque(inserts)
            e_tiles = {}
            pss = [None, None]

            for qt in range(8):
                state[b, qc, qt] = csbp.tile([128, 128], BF16, tag="csb",
                                             name="csb")
            ctx_ps = [ps_tile([128, 260], "cA"), ps_tile([128, 260], "cB")]

            def scores(u, sk, p=None):
                """p=None: full (2x512). p=k: one 256-col piece (alloc on p==0)."""
                if p is None or p == 0:
                    pss[u] = ps_tile([128, 1024], "sA" if u == 0 else "sB")
                cols = ((p * 256, (p + 1) * 256),) if p is not None else \
                    ((0, 512), (512, 1024))
                for lo, hi in cols:
                    nc.tensor.matmul(
                        pss[u][:, lo:hi],
                        KT[u * 64:(u + 1) * 64, sk * 128:(sk + 1) * 128],
                        QT[u * 64:(u + 1) * 64,
                           qc * 1024 + lo: qc * 1024 + hi],
                        start=True, stop=True)

            def alloc_e(u, sk):
                e = expp.tile([128, 1024], BF16, tag=f"e{u}",
                              bufs=(4 if u == 0 else 32), name=f"e{u}_t")
                e_tiles[u, sk] = e
                return e

            def expop(u, sk, p=None):
                e = e_tiles[u, sk] if (u, sk) in e_tiles else alloc_e(u, sk)
                lo, hi = (p * 256, (p + 1) * 256) if p is not None else (0, 1024)
                nc.scalar.activation(e[:, lo:hi], pss[u][:, lo:hi],
                                     Act.Exp, bias=zero_sb, scale=1.0)

            def ctx_mm(u, sk, qt, ps, col0, multigroup=True, first=None):
                # multigroup tiles (cA/cB/u1ps) hold 4 qt groups per bank; a
                # start=True zeroes the whole bank on HW, so either DVE-memset
                # the tile (cA/cB) or pass first=True on exactly the first mm
                # into a fresh bank (u1ps) and accumulate everywhere else.
                if first is None:
                    first = sk == 0 and not multigroup
                nc.tensor.matmul(
                    ps[:, col0:col0 + 65],
                    e_tiles[u, sk][:, qt * 128:(qt + 1) * 128],
                    V[:, sk, u * 65:u * 65 + 65],
                    start=first,
                    stop=(sk == NST - 1),
                    skip_group_check=True)

            def evac(u, qt, ps, col0, on_act=False):
                rcp = rcpp.tile([128, 1], F32, tag="rcp", name="rcp")
                nc.vector.reciprocal(rcp, ps[:, col0 + 64: col0 + 65])
                csb = state[b, qc, qt]
                if on_act:
                    nc.scalar.activation(
                        csb[:, u * 64:(u + 1) * 64], ps[:, col0: col0 + 64],
                        Act.Copy, bias=0.0, scale=rcp)
                else:
                    nc.vector.tensor_scalar(
                        out=csb[:, u * 64:(u + 1) * 64],
                        in0=ps[:, col0: col0 + 64],
                        scalar1=rcp, scalar2=None, op0=Alu.mult)

            def u1_tail_qt(qt, wtag):
                """u1 ctx for one qt group through a w-tag; evac + transpose."""
                ps = ps_tile([128, 512], wtag)
                for sk in range(NST):
                    ctx_mm(1, sk, qt, ps, 0, multigroup=False)
                evac(1, qt, ps, 0)
                csb = state[b, qc, qt]
                sl = slice(qc * 1024 + qt * 128, qc * 1024 + (qt + 1) * 128)
                nc.sync.dma_start_transpose(out=ctxT[:, sl], in_=csb)

            def run_inserts(budget, force_first=False):
                while inserts and (inserts[0][0] <= budget or force_first):
                    force_first = False
                    cost, fn = inserts.popleft()
                    fn()
                    budget -= cost
                return budget

            # -- sk0 scores+exp, optionally in 256-col pieces chasing loads --
            if warm_cb is not None:
                alloc_e(0, 0)
                alloc_e(1, 0)
                for p in range(4):
                    warm_cb(p)
                    scores(0, 0, p=p)
                    expop(0, 0, p=p)
                    scores(1, 0, p=p)
                    expop(1, 0, p=p)
            else:
                scores(0, 0)
                scores(1, 0)
            def drain_u0(sk):
                # first mm into each fresh bank carries start=True (zeroes
                # the whole bank on HW) instead of a DVE memset
                for qt in range(8):
                    half, qtl = divmod(qt, 4)
                    ctx_mm(0, sk, qt, ctx_ps[half], qtl * 65,
                           first=(sk == 0 and qt % 4 == 0))

            u1ps = [None, None]
            for sk in range(NST):
                if not (sk == 0 and warm_cb is not None):
                    expop(0, sk)
                if sk + 1 < NST:
                    scores(0, sk + 1)
                if sk >= 1:
                    # u0 ctx drains one step behind its exp so step-0 inserts
                    # (the first v tiles) can precede it in the PE queue
                    drain_u0(sk - 1)
                fin_batch = final and sk >= 12
                if not fin_batch:
                    rem = run_inserts(520, force_first=True)
                if not (sk == 0 and warm_cb is not None):
                    expop(1, sk)
                if sk + 1 < NST:
                    scores(1, sk + 1)
                if fin_batch:
                    # weave the final qc's u1 ctx (sk 0..14) into the last
                    # loop steps (after this step's scores, so the exp
                    # stream is never blocked behind them): multigroup
                    # accumulation in the w banks (the first mm into each
                    # fresh bank carries start=True and zeroes it), so only
                    # sk15 + the evacuation chains remain after the stream.
                    if sk == 12:
                        u1ps[0] = ps_tile([128, 260], "w1")
                    if sk == 13:
                        u1ps[1] = ps_tile([128, 260], "w2")
                    batch = {12: [(q, k) for q in range(4) for k in range(5)],
                             13: [(q, k) for q in range(4, 8) for k in range(5)],
                             14: [(q, k) for q in range(4) for k in range(5, 10)],
                             15: [(q, k) for q in range(4, 8) for k in range(5, 10)]}
                    for q, k in batch[sk]:
                        half, qtl = divmod(q, 4)
                        ctx_mm(1, k, q, u1ps[half], qtl * 65,
                               first=(q % 4 == 0 and k == 0))
                else:
                    run_inserts(rem + 430)
            drain_u0(NST - 1)
            # u0 normalize+evac (frees cA/cB for the next qc). For the
            # final qc, batch the reciprocals first so the 8 normalize
            # copies then run DVE/ACT in parallel instead of zippering.
            if final:
                rcps = []
                for qt in range(8):
                    half, qtl = divmod(qt, 4)
                    rcp = rcpp.tile([128, 1], F32, tag="rcpf", bufs=8,
                                    name="rcpf")
                    nc.vector.reciprocal(
                        rcp, ctx_ps[half][:, qtl * 65 + 64: qtl * 65 + 65])
                    rcps.append(rcp)
                for qt in range(8):
                    half, qtl = divmod(qt, 4)
                    csb = state[b, qc, qt]
                    src = ctx_ps[half][:, qtl * 65: qtl * 65 + 64]
                    if qt % 2:
                        nc.scalar.activation(csb[:, 0:64], src, Act.Copy,
                                             bias=0.0, scale=rcps[qt])
                    else:
                        nc.vector.tensor_scalar(
                            out=csb[:, 0:64], in0=src,
                            scalar1=rcps[qt], scalar2=None, op0=Alu.mult)
            else:
                for qt in range(8):
                    half, qtl = divmod(qt, 4)
                    evac(0, qt, ctx_ps[half], qtl * 65)

            if not final:
                return list(inserts), \
                    [(450, functools.partial(u1_tail_qt, qt, next_w()))
                     for qt in range(8)]
            # ---- finale: finish u1 (sk10..15) for all qt, batch the
            # reciprocals, then per-qt evac -> transpose -> outproj with
            # DVE/ACT alternation. No HWDGE / DMA sems involved.
            trps = psp.tile([128, 8, 128], BF16, tag="sA", name="trps")
            for qt in range(8):
                half, qtl = divmod(qt, 4)
                for k in range(10, NST):
                    ctx_mm(1, k, qt, u1ps[half], qtl * 65)
                evac(1, qt, u1ps[half], qtl * 65)
                csb = state[b, qc, qt]
                sl = slice(qc * 1024 + qt * 128, qc * 1024 + (qt + 1) * 128)
                nc.tensor.transpose(trps[:, qt, :], csb, ident_sb)
                if qt % 2 == 0:
                    nc.vector.tensor_copy(ctxT[:, sl], trps[:, qt, :])
                else:
                    nc.scalar.activation(ctxT[:, sl], trps[:, qt, :],
                                         Act.Copy, bias=0.0, scale=1.0)
                outproj_st(b, 8 + qt, "cA", "cB", split_evac=True)
            return list(inserts), []

        def thunk(f, *a):
            def g():
                f(*a)
            return g

        # =========================== schedule ===========================
        alloc_qt(0)
        alloc_qt(1)
        nc.sync.dma_start(out=bq_sb, in_=bq_d[:, :])
        nc.sync.dma_start(out=wk_sb, in_=wk_d[:, :])
        qt_chunk(0, 0, ncol=256)
        nc.sync.dma_start(out=wq_sb, in_=wq_d[:, :])
        qt_chunk(0, 256, ncol=256)
        qt_chunk(0, 512, ncol=256)
        qt_chunk(0, 768, ncol=256)
        nc.sync.dma_start(out=wv_sb, in_=wv_d[:, :])
        qt_chunk(0, 1024)
        qt_chunk(0, 1536)
        nc.scalar.dma_start(out=wo_sb, in_=wo_d[:, :])
        nc.scalar.dma_start(out=ident_sb, in_=ident_d[:, :])
        for c0 in range(0, S, 512):
            qt_chunk(1, c0)
        alloc_proj(0)
        alloc_v(0)
        alloc_proj(1)
        alloc_v(1)
        alloc_attn(0)
        alloc_attn(1)

        # prologue: 256-col k/q chunks chase the qt loads; the first exps run
        # as 256-col pieces so the ACT stream starts as early as possible.
        def warm(p):
            if p == 0:
                kq_chunk(0, "k", None, next_w(), c0=0, ncol=256)
                kq_chunk(0, "q", None, next_w(), c0=0, ncol=256)
            elif p == 1:
                kq_chunk(0, "q", None, next_w(), c0=256, ncol=256)
            elif p == 2:
                kq_chunk(0, "k", None, next_w(), c0=256, ncol=256)
                kq_chunk(0, "q", None, next_w(), c0=512, ncol=256)
            else:
                kq_chunk(0, "q", None, next_w(), c0=768, ncol=256)

        def kq_halves(b, which, c):
            w = next_w()
            return [(430, thunk(kq_chunk, b, which, c, w, 0, 4)),
                    (430, thunk(kq_chunk, b, which, c, w, 4, NKT))]

        def v_thunks(b, sts):
            return [(430, thunk(v_st, b, st, next_w())) for st in sts]

        def op_thunks(b, sts):
            # PE cost is only ~430; the DVE evacs ride behind. 500 lets an
            # op share a step with one cheap thunk but not with another op.
            return [(500, thunk(outproj_st, b, st, next_w(), next_w()))
                    for st in sts]

        def interleave(*lists):
            out = []
            ls = [deque(x) for x in lists]
            while any(ls):
                for q in ls:
                    if q:
                        out.append(q.popleft())
            return out

        ins0 = v_thunks(0, (0, 1))
        ins0 += kq_halves(0, "k", 1)
        ins0 += v_thunks(0, (2, 3))
        ins0 += kq_halves(0, "k", 2)
        ins0 += v_thunks(0, (4, 5))
        ins0 += kq_halves(0, "k", 3)
        ins0 += v_thunks(0, (6, 7))
        ins0 += kq_halves(0, "q", 2)
        ins0 += v_thunks(0, (8, 9))
        ins0 += kq_halves(0, "q", 3)
        ins0 += v_thunks(0, (10, 11, 12, 13, 14, 15))
        ins0 += kq_halves(1, "k", 0)
        ins0 += kq_halves(1, "q", 0)
        ins0 += kq_halves(1, "k", 1)
        ins0 += kq_halves(1, "q", 1)
        left, tail0 = attention_qc(0, 0, ins0, warm_cb=warm)

        kq1 = kq_halves(1, "k", 2) + kq_halves(1, "q", 2) + \
            kq_halves(1, "k", 3) + kq_halves(1, "q", 3)
        ins1 = list(left)
        ins1 += interleave(tail0, kq1)
        ins1 += interleave(v_thunks(1, range(2, 10)), op_thunks(0, range(0, 8)))
        left, tail1 = attention_qc(0, 1, ins1)

        # v(b1) first two inline (needed at steps 0/1 of b1-qc0)
        for _, fn in left:
            fn()
        v_st(1, 0, next_w())
        v_st(1, 1, next_w())
        ins2 = list(left)
        ins2 += v_thunks(1, range(10, NST)) + list(tail1)
        ins2 += op_thunks(0, range(8, NST))
        left, tail2 = attention_qc(1, 0, ins2)

        # ins3 must fully drain by step ~10 (the final qc's last steps weave
        # its own u1 ctx through the w banks and take no inserts)
        ins3 = list(left) + list(tail2) + op_thunks(1, range(8))
        left, _ = attention_qc(1, 1, ins3, final=True)
        for _, fn in left:
            fn()

    _split_sync_commands(nc)
    return nc


def _prepare(query, q_w, q_b, k_w, v_w, out_w):
    qt = np.ascontiguousarray(query.reshape(BS, D).T).astype(BF)  # [D, BS]

    def wprep(w, sl):
        # [D, DPC] -> [128, NKT*DPC]: partition = row within k-tile, free =
        # (k, dpc) contiguous, so the load is one fat DMA with 2KB rows.
        wt = np.ascontiguousarray(w[sl, :].T)          # [D, DPC]
        wt = wt.reshape(NKT, 128, DPC).transpose(1, 0, 2).reshape(128, NKT * DPC)
        return np.ascontiguousarray(wt).astype(BF)

    in_maps = []
    for c in range(N_CORES):
        sl = slice(c * DPC, (c + 1) * DPC)
        in_maps.append({
            "qt": qt,
            "wq": wprep(q_w, sl),
            "wk": wprep(k_w, sl),
            "wv": wprep(v_w, sl),
            "bq": np.ascontiguousarray(q_b[sl].reshape(DPC, 1)).astype(np.float32),
            "wo": np.ascontiguousarray(out_w[:, sl].T).astype(BF),
        })
    return in_maps


def kernel(query, mask, q_w, q_b, k_w, k_b, v_w, v_b, out_w, out_b):
    query = np.asarray(query, dtype=np.float32)
    q_w = np.asarray(q_w, dtype=np.float32); q_b = np.asarray(q_b, dtype=np.float32)
    k_w = np.asarray(k_w, dtype=np.float32)
    v_w = np.asarray(v_w, dtype=np.float32); v_b = np.asarray(v_b, dtype=np.float32)
    out_w = np.asarray(out_w, dtype=np.float32); out_b = np.asarray(out_b, dtype=np.float32)
    # k-bias cancels exactly in softmax (adds a per-query constant to all
    # scores of that query). v-bias adds a constant row to ctx (attention
    # rows sum to 1), contributing out_w @ v_b to every output row — folded
    # with out_b on the host.
    in_maps = _prepare(query, q_w, q_b, k_w, v_w, out_w)
    nc = _build()
    res = run_bass_kernel_spmd(nc, in_maps, core_ids=list(range(N_CORES)))
    out = np.zeros((BS, D), dtype=np.float32)
    for c in range(N_CORES):
        out += np.asarray(res.results[c]["out_part"], dtype=np.float32)
    out += (out_b + out_w @ v_b)[None, :]
    return out.reshape(B, S, D)
